# revision 27
# baseline (speedup 1.0000x reference)
"""MoE gate (LLaDA2) routing kernel for 8 Trainium2 NeuronCores.

Strategy: token-parallel over 8 cores (2048 tokens/core).
Router GEMM = fp16 main pass (xhi@whi, [t,e] layout) + BOTH fp32-residual
correction terms (xlo@w + xhi@wlo) computed in ONE fp8 DoubleRow GEMM in
transposed [e,t] layout (w-side stationary, reused across 512-token blocks,
slot0=(w*2^5, xlo*2^11), slot1=(wlo*2^13, xhi*2^3)).  The correction is
folded back into the main PSUM with tiny fp16 identity matmuls
(corr_sb.T @ 2^-14*I after a 2^-2-scaled ACT copy).
Routing epilogue: exact grouped top-8 for indices; weights recovered via a
2^-16-quantized key + 8-bit bias-code mantissa payload, decoded in batched
tail ops (no second top-8 chain).
"""
import sys
for p in ("/opt/trn_rl_repo", "/root/.axon_site/_ro/trn_rl_repo"):
    if p not in sys.path:
        sys.path.append(p)

import numpy as np
import ml_dtypes

T, H, E = 16384, 4096, 256
NCORES = 8
TPC = T // NCORES          # tokens per core: 2048
NTILES = TPC // 128        # 16 row tiles
NBLK = TPC // 512          # 4 token blocks (for fp8 corr GEMM)
KCH = H // 128             # 32 contraction chunks
G = 8                      # expert groups
GS = E // G                # 32 experts/group
K = 8                      # top-k
NEG = -1.0e4
E4 = ml_dtypes.float8_e4m3

_cache = {}


def _build():
    import concourse.bacc as bacc
    import concourse.bass as bass
    import concourse.mybir as mybir
    from concourse import tile

    dt = mybir.dt
    Alu = mybir.AluOpType
    Act = mybir.ActivationFunctionType
    Ax = mybir.AxisListType
    PM = mybir.MatmulPerfMode

    nc = bacc.Bacc("TRN2", target_bir_lowering=False, debug=False,
                   num_devices=NCORES)

    xhi_d = nc.dram_tensor("xhi", [NTILES, 128, KCH, 128], dt.float16, kind="ExternalInput")
    xc_d = nc.dram_tensor("xc", [NBLK, 128, 2, KCH, 512], dt.float8e4, kind="ExternalInput")
    whi_d = nc.dram_tensor("whi", [128, KCH, E], dt.float16, kind="ExternalInput")
    wc_d = nc.dram_tensor("wc", [128, KCH, 2, 2, 128], dt.float8e4, kind="ExternalInput")
    id_d = nc.dram_tensor("ident", [128, 128], dt.float16, kind="ExternalInput")
    tP_d = nc.dram_tensor("btabP", [128, E], dt.float32, kind="ExternalInput")
    tB_d = nc.dram_tensor("ptabB", [128, E], dt.float32, kind="ExternalInput")
    cc_d = nc.dram_tensor("consts", [128, 2], dt.float32, kind="ExternalInput")
    w_out = nc.dram_tensor("w_out", [TPC, K], dt.float32, kind="ExternalOutput")
    i_out = nc.dram_tensor("i_out", [TPC, K], dt.uint32, kind="ExternalOutput")

    with tile.TileContext(nc) as tc:
        with (
            tc.tile_pool(name="wpool", bufs=1) as wpool,
            tc.tile_pool(name="xcpool", bufs=2) as xcpool,
            tc.tile_pool(name="xpool", bufs=5) as xpool,
            tc.tile_pool(name="cpsum", bufs=2, space="PSUM") as cpsum,
            tc.tile_pool(name="mpsum", bufs=4, space="PSUM") as mpsum,
            tc.tile_pool(name="csbp", bufs=2) as csbp,
            tc.tile_pool(name="spool", bufs=4) as spool,
            tc.tile_pool(name="tpool", bufs=4) as tpool,
            tc.tile_pool(name="opool", bufs=1) as opool,
        ):
            whi = wpool.tile([128, KCH * E], dt.float16, tag="whi")
            wc = wpool.tile([128, KCH * 2 * 2 * 128], dt.float8e4, tag="wc")
            ident = wpool.tile([128, 128], dt.float16, tag="ident")
            tP = wpool.tile([128, E], dt.float32, tag="tP")
            tB = wpool.tile([128, E], dt.float32, tag="tB")
            cc = wpool.tile([128, 2], dt.float32, tag="cc")
            # w-side + tables on the scalar HWDGE queue, x-stream on sync:
            # the two rings run concurrently so the first DR matmul isn't
            # starved behind 6MB of serialized loads.
            WCOL = 2 * 2 * 128
            nc.scalar.dma_start(wc[:, :KCH // 2 * WCOL],
                                wc_d[:, :KCH // 2].rearrange("p k h s e -> p (k h s e)"))
            nc.scalar.dma_start(ident[:], id_d[:])
            nc.scalar.dma_start(whi[:], whi_d[:].rearrange("p k e -> p (k e)"))
            nc.scalar.dma_start(wc[:, KCH // 2 * WCOL:],
                                wc_d[:, KCH // 2:].rearrange("p k h s e -> p (k h s e)"))
            nc.scalar.dma_start(tP[:], tP_d[:])
            nc.scalar.dma_start(tB[:], tB_d[:])
            nc.scalar.dma_start(cc[:], cc_d[:])

            out_i = opool.tile([128, NTILES * K], dt.uint32, tag="oi")
            vbB = opool.tile([128, NTILES * K], dt.float32, tag="vbB")
            out_w = opool.tile([128, NTILES * K], dt.float32, tag="ow")
            u = opool.tile([128, NTILES * K], dt.float32, tag="u")
            key = opool.tile([128, NTILES * K], dt.float32, tag="key")
            rB = opool.tile([128, NTILES * K], dt.float32, tag="rB")
            nB = opool.tile([128, NTILES * K], dt.float32, tag="nB")
            mB = opool.tile([128, NTILES * K], dt.float32, tag="mB")
            t1 = opool.tile([128, NTILES * K], dt.float32, tag="t1")
            w8 = opool.tile([128, NTILES * K], dt.float32, tag="w8")
            s16 = opool.tile([128, NTILES], dt.float32, tag="s16")
            rec = opool.tile([128, NTILES], dt.float32, tag="rec")

            wc5 = wc[:].rearrange("p (k h s e) -> p k h s e", k=KCH, h=2, s=2)

            def bc_mid(ap2, n):
                # [128, m] -> [128, n(bcast), m]
                return bass.AP(ap2.tensor, ap2.offset,
                               [list(ap2.ap[0]), [0, n], list(ap2.ap[1])])

            # HAM warmup: ~3.4us of dummy matmuls un-throttle the PE clock
            # (4/8 -> 8/8) while the first input DMAs are still in flight.
            warm = wpool.tile([128, 64], dt.float16, tag="warm")
            nc.gpsimd.memset(warm[:], 0.0)
            wps = mpsum.tile([128, E], dt.float32, tag="ps", name="warmps")
            for i in range(96):
                nc.tensor.matmul(wps[0:64, 0:64], lhsT=warm[:], rhs=warm[:],
                                 start=(i == 0), stop=(i == 95),
                                 skip_group_check=True)

            for b in range(NBLK):
                xc = xcpool.tile([128, 2 * KCH * 512], dt.float8e4, tag="xc")
                # four piecewise loads (slot x k-half) so the first DR matmuls
                # only wait on the first half of the block's data
                KH = KCH // 2
                SC = KCH * 512   # slot stride in sbuf columns
                HC = KH * 512    # k-half stride
                for kh in range(2):
                    for s in range(2):
                        nc.sync.dma_start(
                            xc[:, s * SC + kh * HC: s * SC + (kh + 1) * HC],
                            xc_d[b, :, s, kh * KH:(kh + 1) * KH].rearrange("p k t -> p (k t)"))
                xc4 = xc[:].rearrange("p (s k t) -> p k s t", s=2, k=KCH)

                cps = [cpsum.tile([128, 512], dt.float32, tag=f"cps{eh}",
                                  name=f"cps{eh}") for eh in range(2)]
                def dr_emit(k0, k1):
                    for k in range(k0, k1):
                        xck = xc4[:, k]
                        for eh in range(2):
                            nc.tensor.matmul(cps[eh][:], lhsT=wc5[:, k, eh],
                                             rhs=xck,
                                             start=(k == 0), stop=(k == KCH - 1),
                                             perf_mode=PM.DoubleRow,
                                             skip_group_check=True)

                ps0 = None
                if b == 0:
                    # interleave: DR first half -> tile-0 main matmuls -> DR
                    # second half, so the PE chews whatever data has landed
                    # during the startup DMA window.
                    dr_emit(0, KH)
                    xhi = xpool.tile([128, KCH * 128], dt.float16, tag="xhi")
                    nc.sync.dma_start(xhi[:], xhi_d[0].rearrange("p k t -> p (k t)"))
                    ps0 = mpsum.tile([128, E], dt.float32, tag="ps", name="ps0")
                    for k in range(KCH):
                        nc.tensor.matmul(ps0[:], lhsT=xhi[:, k * 128:(k + 1) * 128],
                                         rhs=whi[:, k * E:(k + 1) * E],
                                         start=(k == 0), stop=False,
                                         skip_group_check=True)
                    dr_emit(KH, KCH)
                else:
                    dr_emit(0, KCH)
                csbs = []
                for j in range(4):
                    pair = []
                    for eh in range(2):
                        cs = csbp.tile([128, 128], dt.float16, tag=f"cs{j}{eh}",
                                       name=f"cs{j}{eh}")
                        nc.scalar.mul(cs[:], cps[eh][:, j * 128:(j + 1) * 128], 0.25)
                        pair.append(cs)
                    csbs.append(pair)

                for j in range(4):
                    jj = 4 * b + j
                    if b == 0 and j == 0:
                        ps = ps0
                    else:
                        xhi = xpool.tile([128, KCH * 128], dt.float16, tag="xhi")
                        nc.sync.dma_start(xhi[:], xhi_d[jj].rearrange("p k t -> p (k t)"))
                        ps = mpsum.tile([128, E], dt.float32, tag="ps", name="ps")
                        for k in range(KCH):
                            nc.tensor.matmul(ps[:], lhsT=xhi[:, k * 128:(k + 1) * 128],
                                             rhs=whi[:, k * E:(k + 1) * E],
                                             start=(k == 0), stop=False)
                    nc.tensor.matmul(ps[:, 0:128], lhsT=csbs[j][0][:], rhs=ident[:],
                                     start=False, stop=False, skip_group_check=True)
                    nc.tensor.matmul(ps[:, 128:256], lhsT=csbs[j][1][:], rhs=ident[:],
                                     start=False, stop=True, skip_group_check=True)

                    # --- routing epilogue ---
                    scores = spool.tile([128, E], dt.float32, tag="scores")
                    nc.scalar.activation(scores[:], ps[:], Act.Sigmoid)
                    sru = spool.tile([128, E], dt.float32, tag="sru")
                    nc.vector.tensor_tensor(sru[:], scores[:], tP[:], Alu.add)
                    smq = spool.tile([128, E], dt.float32, tag="smq")
                    nc.vector.tensor_scalar_add(smq[:], sru[:], 128.0)
                    vB = spool.tile([128, E], dt.float32, tag="vB")
                    nc.vector.scalar_tensor_tensor(vB[:], smq[:], -128.0, tB[:],
                                                   op0=Alu.add, op1=Alu.add)

                    sru3 = sru[:].rearrange("p (g e) -> p g e", g=G)
                    top1 = tpool.tile([128, G], dt.float32, tag="top1")
                    nc.vector.tensor_reduce(top1[:], sru3, axis=Ax.X, op=Alu.max)
                    mr2 = spool.tile([128, E], dt.float32, tag="mr2")
                    nc.vector.match_replace(mr2[:], in_to_replace=top1[:],
                                            in_values=sru[:], imm_value=NEG)
                    top2 = tpool.tile([128, G], dt.float32, tag="top2")
                    nc.vector.tensor_reduce(top2[:], mr2[:].rearrange("p (g e) -> p g e", g=G),
                                            axis=Ax.X, op=Alu.max)
                    gst = tpool.tile([128, G], dt.float32, tag="gs")
                    nc.vector.tensor_tensor(gst[:], top1[:], top2[:], Alu.add)
                    g8 = tpool.tile([128, 8], dt.float32, tag="g8")
                    nc.vector.max(out=g8[:], in_=gst[:])
                    inv = tpool.tile([128, G], dt.float32, tag="inv")
                    nc.vector.tensor_scalar(inv[:], gst[:], g8[:, 3:4], -NEG,
                                            op0=Alu.is_lt, op1=Alu.mult)
                    nc.vector.tensor_tensor(sru3, sru3, inv[:].to_broadcast([128, G, GS]),
                                            Alu.subtract)
                    vB3 = vB[:].rearrange("p (g e) -> p g e", g=G)
                    nc.vector.tensor_tensor(vB3, vB3, inv[:].to_broadcast([128, G, GS]),
                                            Alu.subtract)

                    vals8 = tpool.tile([128, K], dt.float32, tag="vals8")
                    nc.vector.max(out=vals8[:], in_=sru[:])
                    nc.vector.max_index(out=out_i[:, jj * K:(jj + 1) * K],
                                        in_max=vals8[:], in_values=sru[:])
                    nc.vector.max(out=vbB[:, jj * K:(jj + 1) * K], in_=vB[:])

                # --- per-block weight decode on column slice [128, 32] ---
                BK = 4 * K
                sl = slice(b * BK, (b + 1) * BK)
                sl4 = slice(b * 4, (b + 1) * 4)
                nc.vector.tensor_scalar_add(u[:, sl], vbB[:, sl], 128.0)
                nc.vector.tensor_scalar_add(key[:, sl], u[:, sl], -128.0)
                nc.vector.tensor_tensor(rB[:, sl], vbB[:, sl], key[:, sl], Alu.subtract)
                nc.vector.tensor_scalar(nB[:, sl], rB[:, sl], 0.0, 2.0 ** -16,
                                        op0=Alu.is_lt, op1=Alu.mult)
                nc.vector.tensor_tensor(mB[:, sl], nB[:, sl], rB[:, sl], Alu.add)
                nc.vector.scalar_tensor_tensor(t1[:, sl], mB[:, sl], cc[:, 0:1],
                                               vbB[:, sl], op0=Alu.mult, op1=Alu.subtract)
                nc.vector.tensor_scalar(w8[:, sl], t1[:, sl], cc[:, 1:2], -1.0,
                                        op0=Alu.add, op1=Alu.mult)
                w83 = w8[:, sl].rearrange("p (i k) -> p i k", i=4)
                nc.vector.tensor_reduce(s16[:, sl4], w83, axis=Ax.X, op=Alu.add)
                nc.vector.reciprocal(rec[:, sl4], s16[:, sl4])
                ow3 = out_w[:, sl].rearrange("p (i k) -> p i k", i=4)
                nc.vector.scalar_tensor_tensor(ow3, w83, 2.5,
                                               rec[:, sl4].to_broadcast([128, 4, K]),
                                               op0=Alu.mult, op1=Alu.mult)
                nc.scalar.dma_start(w_out[b * 512:(b + 1) * 512].rearrange("(i p) k -> p i k", p=128),
                                    out_w[:, sl].rearrange("p (i k) -> p i k", i=4))
                nc.scalar.dma_start(i_out[b * 512:(b + 1) * 512].rearrange("(i p) k -> p i k", p=128),
                                    out_i[:, sl].rearrange("p (i k) -> p i k", i=4))

    nc.compile()
    return nc


def _prep(hidden_states, weight, expert_bias):
    f32 = np.float32
    x = np.ascontiguousarray(hidden_states, dtype=f32)
    w = np.ascontiguousarray(weight, dtype=f32)
    bias = np.asarray(expert_bias, dtype=f32)

    whi16 = w.astype(np.float16)
    wlo = w - whi16.astype(f32)
    whi_l = np.ascontiguousarray(whi16.reshape(E, KCH, 128).transpose(2, 1, 0))
    w8 = (w * f32(32.0)).astype(E4)
    wlo8 = (wlo * f32(8192.0)).astype(E4)
    wc = np.empty((128, KCH, 2, 2, 128), dtype=E4)
    wc[:, :, :, 0, :] = w8.reshape(2, 128, KCH, 128).transpose(3, 2, 0, 1)
    wc[:, :, :, 1, :] = wlo8.reshape(2, 128, KCH, 128).transpose(3, 2, 0, 1)
    wc = np.ascontiguousarray(wc)

    ident = np.ascontiguousarray((np.eye(128) * 2.0 ** -14).astype(np.float16))

    e_idx = np.arange(E)
    btabP = (bias + ((7 - e_idx // GS) * f32(2.0 ** -22))).astype(f32)
    B0 = f32(bias.min())
    Dl = f32(max(float(bias.max() - bias.min()), 1e-12) / 255.0)
    lev = np.clip(np.round((bias - B0) / Dl), 0, 255).astype(f32)
    ptabB = (lev * f32(2.0 ** -24)).astype(f32)
    bc = lambda a: np.ascontiguousarray(np.broadcast_to(a, (128, E)))
    consts = np.ascontiguousarray(
        np.broadcast_to(np.array([1.0 + float(Dl) * 2.0 ** 24, float(B0)],
                                 dtype=f32), (128, 2)))

    shared = {"whi": whi_l, "wc": wc, "ident": ident, "btabP": bc(btabP),
              "ptabB": bc(ptabB), "consts": consts}

    in_maps = []
    for c in range(NCORES):
        xs = x[c * TPC:(c + 1) * TPC]
        xhi16 = xs.astype(np.float16)
        xlo = xs - xhi16.astype(f32)
        xhi_l = np.ascontiguousarray(
            xhi16.reshape(NTILES, 128, KCH, 128).transpose(0, 3, 2, 1))
        xlo8 = (xlo * f32(2048.0)).astype(E4)
        xhi8 = (xhi16.astype(f32) * f32(8.0)).astype(E4)
        xc = np.ascontiguousarray(np.stack(
            [xlo8.reshape(NBLK, 512, KCH, 128).transpose(0, 3, 2, 1),
             xhi8.reshape(NBLK, 512, KCH, 128).transpose(0, 3, 2, 1)], axis=2))
        m = {"xhi": xhi_l, "xc": xc}
        m.update(shared)
        in_maps.append(m)
    return in_maps


def kernel(hidden_states, weight, expert_bias, _trace=False):
    from concourse.bass_utils import run_bass_kernel_spmd

    if "nc" not in _cache:
        _cache["nc"] = _build()
    nc = _cache["nc"]
    in_maps = _prep(hidden_states, weight, expert_bias)
    res = run_bass_kernel_spmd(nc, in_maps, core_ids=list(range(NCORES)), trace=_trace)
    _cache["last_results"] = res
    w = np.concatenate([res.results[c]["w_out"] for c in range(NCORES)], axis=0)
    idx = np.concatenate([res.results[c]["i_out"] for c in range(NCORES)], axis=0)
    return w.astype(np.float32), idx.astype(np.int32)


# revision 28
# speedup vs baseline: 1.0570x; 1.0570x over previous
"""MoE gate (LLaDA2) routing kernel for 8 Trainium2 NeuronCores.

Strategy: token-parallel over 8 cores (2048 tokens/core).
Router GEMM = fp16 main pass (xhi@whi, [t,e] layout) + BOTH fp32-residual
correction terms (xlo@w + xhi@wlo) computed in ONE fp8 DoubleRow GEMM in
transposed [e,t] layout (w-side stationary, reused across 512-token blocks,
slot0=(w*2^5, xlo*2^11), slot1=(wlo*2^13, xhi*2^3)).  The correction is
folded back into the main PSUM with tiny fp16 identity matmuls
(corr_sb.T @ 2^-14*I after a 2^-2-scaled ACT copy).
Routing epilogue: exact grouped top-8 for indices; weights recovered via a
2^-16-quantized key + 8-bit bias-code mantissa payload, decoded in batched
tail ops (no second top-8 chain).
"""
import sys
for p in ("/opt/trn_rl_repo", "/root/.axon_site/_ro/trn_rl_repo"):
    if p not in sys.path:
        sys.path.append(p)

import numpy as np
import ml_dtypes

T, H, E = 16384, 4096, 256
NCORES = 8
TPC = T // NCORES          # tokens per core: 2048
NTILES = TPC // 128        # 16 row tiles
NBLK = TPC // 512          # 4 token blocks (for fp8 corr GEMM)
KCH = H // 128             # 32 contraction chunks
G = 8                      # expert groups
GS = E // G                # 32 experts/group
K = 8                      # top-k
NEG = -1.0e4
E4 = ml_dtypes.float8_e4m3

_cache = {}


def _build():
    import concourse.bacc as bacc
    import concourse.bass as bass
    import concourse.mybir as mybir
    from concourse import tile

    dt = mybir.dt
    Alu = mybir.AluOpType
    Act = mybir.ActivationFunctionType
    Ax = mybir.AxisListType
    PM = mybir.MatmulPerfMode

    nc = bacc.Bacc("TRN2", target_bir_lowering=False, debug=False,
                   num_devices=NCORES)

    xhi_d = nc.dram_tensor("xhi", [NTILES, 128, KCH, 128], dt.float16, kind="ExternalInput")
    xc_d = nc.dram_tensor("xc", [NBLK, 128, 2, KCH, 512], dt.float8e4, kind="ExternalInput")
    whi_d = nc.dram_tensor("whi", [128, KCH, E], dt.float16, kind="ExternalInput")
    wc_d = nc.dram_tensor("wc", [128, KCH, 2, 2, 128], dt.float8e4, kind="ExternalInput")
    id_d = nc.dram_tensor("ident", [128, 128], dt.float16, kind="ExternalInput")
    tP_d = nc.dram_tensor("btabP", [128, E], dt.float32, kind="ExternalInput")
    tB_d = nc.dram_tensor("ptabB", [128, E], dt.float32, kind="ExternalInput")
    cc_d = nc.dram_tensor("consts", [128, 2], dt.float32, kind="ExternalInput")
    w_out = nc.dram_tensor("w_out", [TPC, K], dt.float32, kind="ExternalOutput")
    i_out = nc.dram_tensor("i_out", [TPC, K], dt.uint32, kind="ExternalOutput")

    with tile.TileContext(nc) as tc:
        with (
            tc.tile_pool(name="wpool", bufs=1) as wpool,
            tc.tile_pool(name="xcpool", bufs=2) as xcpool,
            tc.tile_pool(name="xpool", bufs=3) as xpool,
            tc.tile_pool(name="cpsum", bufs=2, space="PSUM") as cpsum,
            tc.tile_pool(name="mpsum", bufs=4, space="PSUM") as mpsum,
            tc.tile_pool(name="csbp", bufs=2) as csbp,
            tc.tile_pool(name="spool", bufs=3) as spool,
            tc.tile_pool(name="tpool", bufs=3) as tpool,
            tc.tile_pool(name="opool", bufs=1) as opool,
        ):
            whi = wpool.tile([128, KCH * E], dt.float16, tag="whi")
            wc = wpool.tile([128, KCH * 2 * 2 * 128], dt.float8e4, tag="wc")
            ident = wpool.tile([128, 128], dt.float16, tag="ident")
            tP = wpool.tile([128, E], dt.float32, tag="tP")
            tB = wpool.tile([128, E], dt.float32, tag="tB")
            cc = wpool.tile([128, 2], dt.float32, tag="cc")
            # w-side + tables on the scalar HWDGE queue, x-stream on sync:
            # the two rings run concurrently so the first DR matmul isn't
            # starved behind 6MB of serialized loads.
            WCOL = 2 * 2 * 128
            nc.scalar.dma_start(wc[:, :KCH // 2 * WCOL],
                                wc_d[:, :KCH // 2].rearrange("p k h s e -> p (k h s e)"))
            nc.scalar.dma_start(wc[:, KCH // 2 * WCOL:],
                                wc_d[:, KCH // 2:].rearrange("p k h s e -> p (k h s e)"))
            nc.scalar.dma_start(ident[:], id_d[:])
            nc.scalar.dma_start(whi[:], whi_d[:].rearrange("p k e -> p (k e)"))
            nc.scalar.dma_start(tP[:], tP_d[:])
            nc.scalar.dma_start(tB[:], tB_d[:])
            nc.scalar.dma_start(cc[:], cc_d[:])

            out_i = opool.tile([128, NTILES * K], dt.uint32, tag="oi")
            vbB = opool.tile([128, NTILES * K], dt.float32, tag="vbB")
            out_w = opool.tile([128, NTILES * K], dt.float32, tag="ow")
            u = opool.tile([128, NTILES * K], dt.float32, tag="u")
            key = opool.tile([128, NTILES * K], dt.float32, tag="key")
            rB = opool.tile([128, NTILES * K], dt.float32, tag="rB")
            nB = opool.tile([128, NTILES * K], dt.float32, tag="nB")
            mB = opool.tile([128, NTILES * K], dt.float32, tag="mB")
            t1 = opool.tile([128, NTILES * K], dt.float32, tag="t1")
            w8 = opool.tile([128, NTILES * K], dt.float32, tag="w8")
            s16 = opool.tile([128, NTILES], dt.float32, tag="s16")
            rec = opool.tile([128, NTILES], dt.float32, tag="rec")

            wc5 = wc[:].rearrange("p (k h s e) -> p k h s e", k=KCH, h=2, s=2)

            def bc_mid(ap2, n):
                # [128, m] -> [128, n(bcast), m]
                return bass.AP(ap2.tensor, ap2.offset,
                               [list(ap2.ap[0]), [0, n], list(ap2.ap[1])])

            # HAM warmup: ~3.4us of dummy matmuls un-throttle the PE clock
            # (4/8 -> 8/8) while the first input DMAs are still in flight.
            warm = wpool.tile([128, 64], dt.float16, tag="warm")
            nc.gpsimd.memset(warm[:], 0.0)
            wps = mpsum.tile([128, E], dt.float32, tag="ps", name="warmps")
            for i in range(64):
                nc.tensor.matmul(wps[0:64, 0:64], lhsT=warm[:], rhs=warm[:],
                                 start=(i == 0), stop=(i == 63),
                                 skip_group_check=True)

            for b in range(NBLK):
                xc = xcpool.tile([128, 2 * KCH * 512], dt.float8e4, tag="xc")
                # four piecewise loads (slot x k-half) so the first DR matmuls
                # only wait on the first half of the block's data
                KH = KCH // 2
                SC = KCH * 512   # slot stride in sbuf columns
                HC = KH * 512    # k-half stride
                for kh in range(2):
                    for s in range(2):
                        nc.sync.dma_start(
                            xc[:, s * SC + kh * HC: s * SC + (kh + 1) * HC],
                            xc_d[b, :, s, kh * KH:(kh + 1) * KH].rearrange("p k t -> p (k t)"))
                xc4 = xc[:].rearrange("p (s k t) -> p k s t", s=2, k=KCH)

                cps = [cpsum.tile([128, 512], dt.float32, tag=f"cps{eh}",
                                  name=f"cps{eh}") for eh in range(2)]
                def dr_emit(k0, k1):
                    for k in range(k0, k1):
                        xck = xc4[:, k]
                        for eh in range(2):
                            nc.tensor.matmul(cps[eh][:], lhsT=wc5[:, k, eh],
                                             rhs=xck,
                                             start=(k == 0), stop=(k == KCH - 1),
                                             perf_mode=PM.DoubleRow,
                                             skip_group_check=True)

                ps0 = None
                if b == 0:
                    # interleave: DR first half -> tile-0 main matmuls -> DR
                    # second half, so the PE chews whatever data has landed
                    # during the startup DMA window.
                    dr_emit(0, KH)
                    xhi = xpool.tile([128, KCH * 128], dt.float16, tag="xhi")
                    nc.sync.dma_start(xhi[:], xhi_d[0].rearrange("p k t -> p (k t)"))
                    ps0 = mpsum.tile([128, E], dt.float32, tag="ps", name="ps0")
                    for k in range(KCH):
                        nc.tensor.matmul(ps0[:], lhsT=xhi[:, k * 128:(k + 1) * 128],
                                         rhs=whi[:, k * E:(k + 1) * E],
                                         start=(k == 0), stop=False,
                                         skip_group_check=True)
                    dr_emit(KH, KCH)
                else:
                    dr_emit(0, KCH)
                csbs = []
                for j in range(4):
                    pair = []
                    for eh in range(2):
                        cs = csbp.tile([128, 128], dt.float16, tag=f"cs{j}{eh}",
                                       name=f"cs{j}{eh}")
                        nc.scalar.mul(cs[:], cps[eh][:, j * 128:(j + 1) * 128], 0.25)
                        pair.append(cs)
                    csbs.append(pair)

                for j in range(4):
                    jj = 4 * b + j
                    if b == 0 and j == 0:
                        ps = ps0
                    else:
                        xhi = xpool.tile([128, KCH * 128], dt.float16, tag="xhi")
                        nc.sync.dma_start(xhi[:], xhi_d[jj].rearrange("p k t -> p (k t)"))
                        ps = mpsum.tile([128, E], dt.float32, tag="ps", name="ps")
                        for k in range(KCH):
                            nc.tensor.matmul(ps[:], lhsT=xhi[:, k * 128:(k + 1) * 128],
                                             rhs=whi[:, k * E:(k + 1) * E],
                                             start=(k == 0), stop=False)
                    nc.tensor.matmul(ps[:, 0:128], lhsT=csbs[j][0][:], rhs=ident[:],
                                     start=False, stop=False, skip_group_check=True)
                    nc.tensor.matmul(ps[:, 128:256], lhsT=csbs[j][1][:], rhs=ident[:],
                                     start=False, stop=True, skip_group_check=True)

                    # --- routing epilogue ---
                    scores = spool.tile([128, E], dt.float32, tag="scores")
                    nc.scalar.activation(scores[:], ps[:], Act.Sigmoid)
                    sru = spool.tile([128, E], dt.float32, tag="sru")
                    nc.vector.tensor_tensor(sru[:], scores[:], tP[:], Alu.add)
                    smq = spool.tile([128, E], dt.float32, tag="smq")
                    nc.vector.tensor_scalar_add(smq[:], sru[:], 128.0)
                    vB = spool.tile([128, E], dt.float32, tag="vB")
                    nc.vector.scalar_tensor_tensor(vB[:], smq[:], -128.0, tB[:],
                                                   op0=Alu.add, op1=Alu.add)

                    sru3 = sru[:].rearrange("p (g e) -> p g e", g=G)
                    top1 = tpool.tile([128, G], dt.float32, tag="top1")
                    nc.vector.tensor_reduce(top1[:], sru3, axis=Ax.X, op=Alu.max)
                    mr2 = spool.tile([128, E], dt.float32, tag="mr2")
                    nc.vector.match_replace(mr2[:], in_to_replace=top1[:],
                                            in_values=sru[:], imm_value=NEG)
                    top2 = tpool.tile([128, G], dt.float32, tag="top2")
                    nc.vector.tensor_reduce(top2[:], mr2[:].rearrange("p (g e) -> p g e", g=G),
                                            axis=Ax.X, op=Alu.max)
                    gst = tpool.tile([128, G], dt.float32, tag="gs")
                    nc.vector.tensor_tensor(gst[:], top1[:], top2[:], Alu.add)
                    g8 = tpool.tile([128, 8], dt.float32, tag="g8")
                    nc.vector.max(out=g8[:], in_=gst[:])
                    inv = tpool.tile([128, G], dt.float32, tag="inv")
                    nc.vector.tensor_scalar(inv[:], gst[:], g8[:, 3:4], -NEG,
                                            op0=Alu.is_lt, op1=Alu.mult)
                    nc.vector.tensor_tensor(sru3, sru3, inv[:].to_broadcast([128, G, GS]),
                                            Alu.subtract)
                    vB3 = vB[:].rearrange("p (g e) -> p g e", g=G)
                    nc.vector.tensor_tensor(vB3, vB3, inv[:].to_broadcast([128, G, GS]),
                                            Alu.subtract)

                    vals8 = tpool.tile([128, K], dt.float32, tag="vals8")
                    nc.vector.max(out=vals8[:], in_=sru[:])
                    nc.vector.max_index(out=out_i[:, jj * K:(jj + 1) * K],
                                        in_max=vals8[:], in_values=sru[:])
                    nc.vector.max(out=vbB[:, jj * K:(jj + 1) * K], in_=vB[:])

                # --- per-block weight decode on column slice [128, 32] ---
                BK = 4 * K
                sl = slice(b * BK, (b + 1) * BK)
                sl4 = slice(b * 4, (b + 1) * 4)
                nc.vector.tensor_scalar_add(u[:, sl], vbB[:, sl], 128.0)
                nc.vector.tensor_scalar_add(key[:, sl], u[:, sl], -128.0)
                nc.vector.tensor_tensor(rB[:, sl], vbB[:, sl], key[:, sl], Alu.subtract)
                nc.vector.tensor_scalar(nB[:, sl], rB[:, sl], 0.0, 2.0 ** -16,
                                        op0=Alu.is_lt, op1=Alu.mult)
                nc.vector.tensor_tensor(mB[:, sl], nB[:, sl], rB[:, sl], Alu.add)
                nc.vector.scalar_tensor_tensor(t1[:, sl], mB[:, sl], cc[:, 0:1],
                                               vbB[:, sl], op0=Alu.mult, op1=Alu.subtract)
                nc.vector.tensor_scalar(w8[:, sl], t1[:, sl], cc[:, 1:2], -1.0,
                                        op0=Alu.add, op1=Alu.mult)
                w83 = w8[:, sl].rearrange("p (i k) -> p i k", i=4)
                nc.vector.tensor_reduce(s16[:, sl4], w83, axis=Ax.X, op=Alu.add)
                nc.vector.reciprocal(rec[:, sl4], s16[:, sl4])
                ow3 = out_w[:, sl].rearrange("p (i k) -> p i k", i=4)
                nc.vector.scalar_tensor_tensor(ow3, w83, 2.5,
                                               rec[:, sl4].to_broadcast([128, 4, K]),
                                               op0=Alu.mult, op1=Alu.mult)
                nc.scalar.dma_start(w_out[b * 512:(b + 1) * 512].rearrange("(i p) k -> p i k", p=128),
                                    out_w[:, sl].rearrange("p (i k) -> p i k", i=4))
                nc.scalar.dma_start(i_out[b * 512:(b + 1) * 512].rearrange("(i p) k -> p i k", p=128),
                                    out_i[:, sl].rearrange("p (i k) -> p i k", i=4))

    nc.compile()
    return nc


def _prep(hidden_states, weight, expert_bias):
    f32 = np.float32
    x = np.ascontiguousarray(hidden_states, dtype=f32)
    w = np.ascontiguousarray(weight, dtype=f32)
    bias = np.asarray(expert_bias, dtype=f32)

    whi16 = w.astype(np.float16)
    wlo = w - whi16.astype(f32)
    whi_l = np.ascontiguousarray(whi16.reshape(E, KCH, 128).transpose(2, 1, 0))
    w8 = (w * f32(32.0)).astype(E4)
    wlo8 = (wlo * f32(8192.0)).astype(E4)
    wc = np.empty((128, KCH, 2, 2, 128), dtype=E4)
    wc[:, :, :, 0, :] = w8.reshape(2, 128, KCH, 128).transpose(3, 2, 0, 1)
    wc[:, :, :, 1, :] = wlo8.reshape(2, 128, KCH, 128).transpose(3, 2, 0, 1)
    wc = np.ascontiguousarray(wc)

    ident = np.ascontiguousarray((np.eye(128) * 2.0 ** -14).astype(np.float16))

    e_idx = np.arange(E)
    btabP = (bias + ((7 - e_idx // GS) * f32(2.0 ** -22))).astype(f32)
    B0 = f32(bias.min())
    Dl = f32(max(float(bias.max() - bias.min()), 1e-12) / 255.0)
    lev = np.clip(np.round((bias - B0) / Dl), 0, 255).astype(f32)
    ptabB = (lev * f32(2.0 ** -24)).astype(f32)
    bc = lambda a: np.ascontiguousarray(np.broadcast_to(a, (128, E)))
    consts = np.ascontiguousarray(
        np.broadcast_to(np.array([1.0 + float(Dl) * 2.0 ** 24, float(B0)],
                                 dtype=f32), (128, 2)))

    shared = {"whi": whi_l, "wc": wc, "ident": ident, "btabP": bc(btabP),
              "ptabB": bc(ptabB), "consts": consts}

    in_maps = []
    for c in range(NCORES):
        xs = x[c * TPC:(c + 1) * TPC]
        xhi16 = xs.astype(np.float16)
        xlo = xs - xhi16.astype(f32)
        xhi_l = np.ascontiguousarray(
            xhi16.reshape(NTILES, 128, KCH, 128).transpose(0, 3, 2, 1))
        xlo8 = (xlo * f32(2048.0)).astype(E4)
        xhi8 = (xhi16.astype(f32) * f32(8.0)).astype(E4)
        xc = np.ascontiguousarray(np.stack(
            [xlo8.reshape(NBLK, 512, KCH, 128).transpose(0, 3, 2, 1),
             xhi8.reshape(NBLK, 512, KCH, 128).transpose(0, 3, 2, 1)], axis=2))
        m = {"xhi": xhi_l, "xc": xc}
        m.update(shared)
        in_maps.append(m)
    return in_maps


def kernel(hidden_states, weight, expert_bias, _trace=False):
    from concourse.bass_utils import run_bass_kernel_spmd

    if "nc" not in _cache:
        _cache["nc"] = _build()
    nc = _cache["nc"]
    in_maps = _prep(hidden_states, weight, expert_bias)
    res = run_bass_kernel_spmd(nc, in_maps, core_ids=list(range(NCORES)), trace=_trace)
    _cache["last_results"] = res
    w = np.concatenate([res.results[c]["w_out"] for c in range(NCORES)], axis=0)
    idx = np.concatenate([res.results[c]["i_out"] for c in range(NCORES)], axis=0)
    return w.astype(np.float32), idx.astype(np.int32)
